# revision 23
# baseline (speedup 1.0000x reference)
"""Trainium2 Bass kernel for nn_Attention_8323646620215.

LayerNorm -> QKV -> scores(+rel-bias+mask) -> softmax -> attn@V -> out proj.

Sharding: 8 cores = (batch b in 0..3) x (query-half in 0..1). Each core
computes the full K/V for its batch and attention for its 1024 query rows;
no cross-core communication. Inside a core everything is computed in
transposed layouts so no on-chip transposes are needed beyond one PE
transpose of the LayerNorm output:

  xn_T[feat, tok]  (PE transpose)
  QT/KT[hd, tok] = Wqkv.T-slices @ xn_T      (scores contraction over hd=64)
  V[tok, hd]     = xn_T-slices.T @ Wqkv-v
  S_T[j, i]      = KT-slice.T @ QT           (psum, f32)
  P_T            = exp(S_T) * expA           (ACT exp, DVE mult)
  att_T[hd+1, i] = [V|1].T @ P_T             (row 64 = softmax denominator)
  y[i, :]        = sum_h outT_h-slice.T @ Wout-rows + bout

The mask+relative-bias enter multiplicatively: exp(s + bias + m) =
exp(s) * expA with expA = exp(clip-bias) * (mask != 0), and exp(clip-bias)
is generated on-chip from a tiny 3072-entry table via a flipped-identity
matmul (Toeplitz expansion). LN gamma / attention scale / beta are folded
into the QKV weights host-side (exact transforms).

The host permutes each core's tokens so its own query half comes first,
which keeps the device program identical across cores (single NEFF).
"""
import sys
import types
import numpy as np

sys.path.insert(0, "/opt/trn_rl_repo")

# ---- environment fixes (axon agent container) -------------------------------
if "antenv.axon_hooks" not in sys.modules:
    _m = types.ModuleType("antenv.axon_hooks")
    _m._hook = None
    _m.set_axon_ntff_profile_hook = lambda h: setattr(_m, "_hook", h)
    _m.get_axon_ntff_profile_hook = lambda: _m._hook
    sys.modules["antenv.axon_hooks"] = _m
    try:
        from trn_agent_boot.trn_boot import _ntff_profile_via_ctypes
        _m._hook = _ntff_profile_via_ctypes("/opt/axon/libaxon_pjrt.so")
    except Exception:
        pass

import ml_dtypes  # noqa: E402
from concourse import bass, mybir, tile  # noqa: E402
from concourse.bass_utils import run_bass_kernel_spmd  # noqa: E402
from concourse.masks import make_identity  # noqa: E402

F32 = mybir.dt.float32
BF16 = mybir.dt.bfloat16
AF = mybir.ActivationFunctionType
OP = mybir.AluOpType

B, N, D, H, DH, MAXREL = 4, 2048, 512, 8, 64, 200
NQ = N // 2          # queries per core
NT = N // 128        # 16 token tiles
NCORES = 8

# This container's walrus rejects instructions with more than one sem wait.
# Splitting is sound: a same-engine NoOp right before the instruction
# enforces the wait at the same program point (sequencers run in order).


def _split_waits(nc, maxw=1):
    n_split = 0
    for f in nc.m.functions:
        for blk in f.blocks:
            bb = blk.bb if hasattr(blk, "bb") else blk
            insts = list(bb.instructions)
            out = []
            changed = False
            for inst in insts:
                si = inst.sync_info
                waits = list(si.on_wait) if si and si.on_wait else []
                if len(waits) > maxw:
                    extra = waits[:-maxw]
                    chunks = [extra[j:j + maxw] for j in range(0, len(extra), maxw)]
                    for i, chunk in enumerate(chunks):
                        nop = mybir.InstNoOp(name=f"{inst.name}-ws{i}", ins=[], outs=[])
                        nop.engine = inst.engine
                        nop.sync_info = mybir.SyncInfo(on_wait=chunk, on_update=[])
                        out.append(nop)
                    si.on_wait = waits[-maxw:]
                    changed = True
                    n_split += 1
                out.append(inst)
            if changed:
                bb.instructions = out
    return n_split


def build(has_c=False):
    nc = bass.Bass("TRN2", target_bir_lowering=False, debug=False,
                   num_devices=NCORES)
    x_d = nc.dram_tensor("x", [N, D], F32, kind="ExternalInput")
    wqkv_d = nc.dram_tensor("wqkv", [D, 3 * D], BF16, kind="ExternalInput")
    cqkv_d = nc.dram_tensor("cqkv", [3 * D], F32, kind="ExternalInput")
    wout_d = nc.dram_tensor("wout", [D, D], BF16, kind="ExternalInput")
    bout_d = nc.dram_tensor("bout", [D], F32, kind="ExternalInput")
    m01t_d = nc.dram_tensor("m01t", [N, NQ], BF16, kind="ExternalInput")
    expa_d = nc.dram_tensor("expta", [3072], BF16, kind="ExternalInput")
    expb_d = nc.dram_tensor("exptb", [3072], BF16, kind="ExternalInput")
    jrev_d = nc.dram_tensor("jrev", [128, 128], BF16, kind="ExternalInput")
    dsb_d = nc.dram_tensor("den_scratch", [H, NQ], BF16)
    dsi_d = nc.dram_tensor("invden_scratch", [H, NQ], BF16)
    y_d = nc.dram_tensor("y", [NQ, D], F32, kind="ExternalOutput")

    with tile.TileContext(nc) as tc, \
         tc.tile_pool(name="const", bufs=1) as C, \
         tc.tile_pool(name="pers", bufs=1) as P, \
         tc.tile_pool(name="work", bufs=3) as W:

        # ---- persistent tiles ----------------------------------------------
        xnT = [P.tile([128, N], BF16, tag=f"xnT{fb}", name=f"xnT{fb}") for fb in range(4)]
        KTp = [P.tile([128, N], BF16, tag=f"KT{hp}", name=f"KT{hp}") for hp in range(4)]
        QTp = [P.tile([128, NQ], BF16, tag=f"QT{hp}", name=f"QT{hp}") for hp in range(4)]
        Vau = [P.tile([128, H, 66], BF16, tag=f"V{t}", name=f"Vau{t}") for t in range(NT)]
        expA = [P.tile([128, NQ], BF16, tag=f"eA{t}", name=f"eA{t}") for t in range(NT)]
        numT = [P.tile([65, NQ], BF16, tag=f"nT{h}", name=f"nT{h}") for h in range(H)]
        pairT = [P.tile([128, NQ], BF16, tag=f"pT{hp}", name=f"pT{hp}") for hp in range(4)]
        g_all = [P.tile([128, 2944], BF16, tag=f"gall{v}", name=f"gall{v}") for v in range(2)]

        # ---- input DMAs: x tiles first on sync (HWDGE), then mask tiles;
        # weights/tables ride the gpsimd (SWDGE) queues in parallel.
        x_ts = [W.tile([128, D], F32, tag="x", bufs=4, name=f"x{t}") for t in range(NT)]
        for t in range(NT):
            nc.sync.dma_start(out=x_ts[t][:], in_=x_d[t * 128:(t + 1) * 128, :])
        m_ts = [W.tile([128, NQ], BF16, tag="m01", bufs=4, name=f"m01_{t}") for t in range(NT)]
        for t in range(NT):
            nc.sync.dma_start(out=m_ts[t][:], in_=m01t_d[t * 128:(t + 1) * 128, :])

        ident = C.tile([128, 128], BF16, tag="ident")
        make_identity(nc, ident[:])
        jrev = C.tile([128, 128], BF16, tag="jrev")
        nc.gpsimd.dma_start(out=jrev[:], in_=jrev_d[:, :])
        eps_t = C.tile([128, 1], F32, tag="eps")
        nc.vector.memset(eps_t[:], 1e-5)
        wqkv_sb = [C.tile([128, 3 * D], BF16, tag=f"wqkv{kb}", name=f"wqkv{kb}") for kb in range(4)]
        for kb in range(4):
            nc.gpsimd.dma_start(out=wqkv_sb[kb][:],
                                in_=wqkv_d[kb * 128:(kb + 1) * 128, :])
        for v, tab in enumerate((expa_d, expb_d)):
            nc.gpsimd.dma_start(
                out=g_all[v][:],
                in_=bass.AP(tensor=tab.ap().tensor, offset=0,
                            ap=[[1, 128], [1, 2944]]))
        cq_all = C.tile([128, 12], F32, tag="cq")
        nc.gpsimd.dma_start(
            out=cq_all[:],
            in_=bass.AP(tensor=cqkv_d.ap().tensor, offset=0,
                        ap=[[1, 128], [128, 12]]))
        cv_bc = C.tile([128, D], F32, tag="cv")
        nc.gpsimd.dma_start(
            out=cv_bc[:],
            in_=bass.AP(tensor=cqkv_d.ap().tensor, offset=2 * D,
                        ap=[[0, 128], [1, D]]))
        bout_bc = C.tile([128, D], F32, tag="bout")
        nc.gpsimd.dma_start(
            out=bout_bc[:],
            in_=bass.AP(tensor=bout_d.ap().tensor, offset=0,
                        ap=[[0, 128], [1, D]]))
        woutP = [C.tile([128, D], BF16, tag=f"woutP{hp}", name=f"woutP{hp}") for hp in range(4)]
        for hp in range(4):
            nc.gpsimd.dma_start(out=woutP[hp][:],
                                in_=wout_d[hp * 128:(hp + 1) * 128, :])

        # ---- Phase A: LayerNorm + transpose, software-pipelined so the DVE
        # never sits in-FIFO behind the ACT sqrt round trip. -----------------
        with tc.tile_pool(name="psA", bufs=4, space="PSUM") as psA:
            mvs = [None] * NT
            rss = [None] * NT

            def ln_stats(t):
                st = W.tile([128, 6], F32, tag="st")
                nc.vector.bn_stats(out=st[:], in_=x_ts[t][:])
                mv = W.tile([128, 2], F32, tag="mv", bufs=4, name=f"mv{t}")
                nc.vector.bn_aggr(out=mv[:], in_=st[:])
                rs = W.tile([128, 1], F32, tag="rs", bufs=4, name=f"rs{t}")
                nc.scalar.activation(out=rs[:], in_=mv[:, 1:2], func=AF.Sqrt,
                                     bias=eps_t[:])
                mvs[t], rss[t] = mv, rs

            def ln_apply(t):
                nc.vector.reciprocal(out=rss[t][:], in_=rss[t][:])
                xn_bf = W.tile([128, D], BF16, tag="xn")
                nc.vector.tensor_scalar(out=xn_bf[:], in0=x_ts[t][:],
                                        scalar1=mvs[t][:, 0:1],
                                        scalar2=rss[t][:],
                                        op0=OP.subtract, op1=OP.mult)
                for fb in range(4):
                    tp = psA.tile([128, 128], BF16, tag="tr")
                    nc.tensor.transpose(tp[:], xn_bf[:, fb * 128:(fb + 1) * 128],
                                        ident[:])
                    if fb % 2 == 0:
                        nc.vector.tensor_copy(
                            out=xnT[fb][:, t * 128:(t + 1) * 128], in_=tp[:])
                    else:
                        nc.scalar.copy(
                            out=xnT[fb][:, t * 128:(t + 1) * 128], in_=tp[:])
                if t >= 2:
                    # hold the PE clock warm through the LN phase so the
                    # dense QKV section starts at 2.4GHz (results unread)
                    for wu in range(3):
                        dmy = psA.tile([128, 512], F32, tag="dmyA", bufs=2,
                                       name=f"dmyA{t}_{wu}")
                        nc.tensor.matmul(dmy[:], wqkv_sb[0][:, 0:128],
                                         wqkv_sb[1][:, 0:512],
                                         start=True, stop=True)

            ln_stats(0)
            for t in range(1, NT):
                ln_stats(t)
                ln_apply(t - 1)
            ln_apply(NT - 1)

        # ---- Phase B: expA tiles first (PE+DVE), then QKV (PE+ACT/DVE) -----
        with tc.tile_pool(name="psB", bufs=2, space="PSUM") as psB:
            for t in range(NT):
                v = 0 if t < 8 else 1
                goff = 1920 - 128 * (t % 8)
                bp = psB.tile([128, NQ], F32, tag="bps")
                for ic in range(2):
                    nc.tensor.matmul(
                        bp[:, ic * 512:(ic + 1) * 512], jrev[:],
                        g_all[v][:, goff + ic * 512:goff + (ic + 1) * 512],
                        start=True, stop=True)
                nc.vector.tensor_mul(out=expA[t][:], in0=bp[:], in1=m_ts[t][:])
            for hp in range(4):
                for ic in range(2):
                    qp = psB.tile([128, 512], F32, tag="qkvps")
                    for kb in range(4):
                        nc.tensor.matmul(
                            qp[:],
                            wqkv_sb[kb][:, hp * 128:(hp + 1) * 128],
                            xnT[kb][:, ic * 512:(ic + 1) * 512],
                            start=(kb == 0), stop=(kb == 3))
                    if has_c:
                        nc.vector.tensor_scalar_add(
                            out=QTp[hp][:, ic * 512:(ic + 1) * 512], in0=qp[:],
                            scalar1=cq_all[:, hp:hp + 1])
                    else:
                        nc.scalar.copy(
                            out=QTp[hp][:, ic * 512:(ic + 1) * 512], in_=qp[:])
            for hp in range(4):
                for tc4 in range(4):
                    kp = psB.tile([128, 512], F32, tag="qkvps")
                    for kb in range(4):
                        nc.tensor.matmul(
                            kp[:],
                            wqkv_sb[kb][:, D + hp * 128:D + (hp + 1) * 128],
                            xnT[kb][:, tc4 * 512:(tc4 + 1) * 512],
                            start=(kb == 0), stop=(kb == 3))
                    if has_c:
                        nc.vector.tensor_scalar_add(
                            out=KTp[hp][:, tc4 * 512:(tc4 + 1) * 512], in0=kp[:],
                            scalar1=cq_all[:, 4 + hp:5 + hp])
                    else:
                        nc.scalar.copy(
                            out=KTp[hp][:, tc4 * 512:(tc4 + 1) * 512], in_=kp[:])
            for t in range(NT):
                vp = psB.tile([128, 512], F32, tag="qkvps")
                for kb in range(4):
                    nc.tensor.matmul(
                        vp[:],
                        xnT[kb][:, t * 128:(t + 1) * 128],
                        wqkv_sb[kb][:, 2 * D:3 * D],
                        start=(kb == 0), stop=(kb == 3))
                nc.vector.memset(Vau[t][:, :, 64:65], 1.0)
                if has_c:
                    nc.vector.tensor_add(out=Vau[t][:, :, 0:64], in0=vp[:],
                                         in1=cv_bc[:])
                else:
                    nc.vector.tensor_copy(out=Vau[t][:, :, 0:64], in_=vp[:])

        # ---- Phase C: attention --------------------------------------------
        def den_pieces(hp):
            """Normalization of pair hp's heads, as a list of small closures
            interleaved into the next pair's iterations (all DVE + DMA)."""
            dal = W.tile([128, 2 * NQ // 128], BF16, tag="dall", bufs=2,
                         name=f"dal{hp}")
            dbs = [None, None]

            def p0():
                nc.sync.dma_start(
                    out=dal[:],
                    in_=bass.AP(tensor=dsb_d.ap().tensor, offset=2 * hp * NQ,
                                ap=[[2 * NQ // 128, 128], [1, 2 * NQ // 128]]))

            def p1():
                nc.vector.tensor_scalar_add(out=dal[:], in0=dal[:],
                                            scalar1=1e-20)
                with nc.allow_low_precision(reason="bf16 softmax denominators"):
                    nc.vector.reciprocal(out=dal[:], in_=dal[:])
                nc.sync.dma_start(
                    out=bass.AP(tensor=dsi_d.ap().tensor, offset=2 * hp * NQ,
                                ap=[[2 * NQ // 128, 128], [1, 2 * NQ // 128]]),
                    in_=dal[:])

            def load_bc(e):
                def f():
                    h = 2 * hp + e
                    den_bc = W.tile([64, NQ], BF16, tag="denb", bufs=2,
                                    name=f"denb{h}")
                    dbs[e] = den_bc
                    nc.sync.dma_start(
                        out=den_bc[:],
                        in_=bass.AP(tensor=dsi_d.ap().tensor, offset=h * NQ,
                                    ap=[[0, 64], [1, NQ]]))
                return f

            def mul_chunk(e, half):
                def f():
                    h = 2 * hp + e
                    sl = slice(half * 512, (half + 1) * 512)
                    if e == 0:
                        nc.vector.tensor_mul(out=pairT[hp][0:64, sl],
                                             in0=numT[h][0:64, sl],
                                             in1=dbs[e][:, sl])
                    else:
                        nc.vector.tensor_mul(out=numT[h][0:64, sl],
                                             in0=numT[h][0:64, sl],
                                             in1=dbs[e][:, sl])
                return f

            def stitch():
                nc.sync.dma_start(out=pairT[hp][64:128, :],
                                  in_=numT[2 * hp + 1][0:64, :])

            return [p0, p1, load_bc(0), mul_chunk(0, 0), mul_chunk(0, 1),
                    load_bc(1), mul_chunk(1, 0), mul_chunk(1, 1), stitch]

        with tc.tile_pool(name="psC", bufs=2, space="PSUM") as psC:
            pend = []
            for hp in range(4):
                h0, h1 = 2 * hp, 2 * hp + 1
                av0 = psC.tile([65, NQ], F32, tag="av0", name="av0", bufs=1)
                av1 = psC.tile([65, NQ], F32, tag="av1", name="av1", bufs=1)
                for jt in range(NT):
                    for ic in range(2):
                        i5 = ic * 512
                        sp = psC.tile([128, 1024], F32, tag="sp")
                        nc.tensor.matmul(
                            sp[:, 0:512],
                            KTp[hp][0:64, jt * 128:(jt + 1) * 128],
                            QTp[hp][0:64, i5:i5 + 512],
                            start=True, stop=True, tile_position=(0, 0))
                        nc.tensor.matmul(
                            sp[:, 512:1024],
                            KTp[hp][64:128, jt * 128:(jt + 1) * 128],
                            QTp[hp][64:128, i5:i5 + 512],
                            start=True, stop=True, tile_position=(64, 0))
                        eb = W.tile([128, 1024], BF16, tag="eb")
                        nc.scalar.activation(out=eb[:], in_=sp[:], func=AF.Exp)
                        pb = W.tile([128, 1024], BF16, tag="pb")
                        nc.vector.tensor_mul(out=pb[:, 0:512], in0=eb[:, 0:512],
                                             in1=expA[jt][:, i5:i5 + 512])
                        nc.vector.tensor_mul(out=pb[:, 512:1024],
                                             in0=eb[:, 512:1024],
                                             in1=expA[jt][:, i5:i5 + 512])
                        nc.tensor.matmul(av0[:, i5:i5 + 512],
                                         Vau[jt][:, h0, 0:65], pb[:, 0:512],
                                         start=(jt == 0), stop=(jt == NT - 1))
                        nc.tensor.matmul(av1[:, i5:i5 + 512],
                                         Vau[jt][:, h1, 0:65], pb[:, 512:1024],
                                         start=(jt == 0), stop=(jt == NT - 1))
                        if pend and jt >= 2:
                            pend.pop(0)()
                        if jt == NT - 1:
                            nc.vector.tensor_copy(
                                out=numT[h0][:, i5:i5 + 512],
                                in_=av0[:, i5:i5 + 512])
                            nc.scalar.copy(
                                out=numT[h1][:, i5:i5 + 512],
                                in_=av1[:, i5:i5 + 512])
                for e in range(2):
                    h = 2 * hp + e
                    nc.sync.dma_start(out=dsb_d[h, :], in_=numT[h][64:65, :])
                pend = den_pieces(hp)
            for f in pend:
                f()

        # ---- Phase D: output projection (head pairs, K=128) ----------------
        with tc.tile_pool(name="psD", bufs=1, space="PSUM") as psD:
            yps = [psD.tile([128, 512], F32, tag=f"yp{isl}", name=f"yp{isl}")
                   for isl in range(8)]
            for hp in range(4):
                for isl in range(8):
                    nc.tensor.matmul(yps[isl][:],
                                     pairT[hp][:, isl * 128:(isl + 1) * 128],
                                     woutP[hp][:],
                                     start=(hp == 0), stop=(hp == 3))
            for isl in range(8):
                ysb = W.tile([128, 512], F32, tag="ysb", bufs=2)
                nc.vector.tensor_add(out=ysb[:], in0=yps[isl][:],
                                     in1=bout_bc[:])
                nc.sync.dma_start(out=y_d[isl * 128:(isl + 1) * 128, :],
                                  in_=ysb[:])
    _split_waits(nc)
    return nc


_NC_CACHE = {}


def _get_nc(has_c):
    if has_c not in _NC_CACHE:
        _NC_CACHE[has_c] = build(has_c)
    return _NC_CACHE[has_c]


LAST_EXEC_TIME_NS = None


def kernel(x, gamma, beta, Wqkv, Wout, bout, rel_table, temporal_mask,
           trace=True):
    global LAST_EXEC_TIME_NS
    x = np.asarray(x, np.float32)
    gamma = np.asarray(gamma, np.float32)
    beta = np.asarray(beta, np.float32)
    Wqkv = np.asarray(Wqkv, np.float32)
    Wout = np.asarray(Wout, np.float32)
    bout = np.asarray(bout, np.float32)
    rel_table = np.asarray(rel_table, np.float32)
    temporal_mask = np.asarray(temporal_mask)

    scale = DH ** -0.5
    w_eff = (Wqkv * gamma[:, None]).copy()
    w_eff[:, :D] *= scale
    cqkv = (beta @ Wqkv).astype(np.float32)
    cqkv[:D] *= scale
    wqkv_bf = w_eff.astype(ml_dtypes.bfloat16)
    wout_bf = Wout.astype(ml_dtypes.bfloat16)
    mask01T = (temporal_mask != 0).astype(np.float32).T  # [key j, query i]

    def expbias(d):
        idx = np.clip(d, -(MAXREL - 1), MAXREL - 1) + MAXREL - 1
        return np.exp(rel_table[idx])

    w = np.arange(3072)
    expt_a = expbias(w - 2047).astype(ml_dtypes.bfloat16)
    expt_b_half = [
        expbias(w - 2047 + 2 * (half * NQ) - NQ).astype(ml_dtypes.bfloat16)
        for half in range(2)
    ]
    keyperm_half = [
        np.concatenate([np.arange(i0, i0 + NQ),
                        np.arange(NQ - i0, NQ - i0 + NQ)])
        for i0 in (0, NQ)
    ]
    m01t_half = [
        np.ascontiguousarray(
            mask01T[keyperm_half[half]][:, half * NQ:(half + 1) * NQ]
        ).astype(ml_dtypes.bfloat16)
        for half in range(2)
    ]
    jrev_np = np.eye(128, dtype=np.float32)[::-1].astype(ml_dtypes.bfloat16).copy()

    in_maps = []
    for c in range(NCORES):
        b, half = c // 2, c % 2
        xp = np.ascontiguousarray(x[b][keyperm_half[half]])
        in_maps.append({
            "x": xp,
            "wqkv": wqkv_bf,
            "cqkv": cqkv,
            "wout": wout_bf,
            "bout": bout,
            "m01t": m01t_half[half],
            "expta": expt_a,
            "exptb": expt_b_half[half],
            "jrev": jrev_np,
        })

    nc = _get_nc(bool(np.any(cqkv != 0.0)))
    res = run_bass_kernel_spmd(nc, in_maps, core_ids=list(range(NCORES)),
                               trace=trace)
    LAST_EXEC_TIME_NS = res.exec_time_ns

    out = np.empty((B, N, D), np.float32)
    for c in range(NCORES):
        b, half = c // 2, c % 2
        out[b, half * NQ:(half + 1) * NQ] = res.results[c]["y"]
    return out


# revision 24
# speedup vs baseline: 1.0376x; 1.0376x over previous
"""Trainium2 Bass kernel for nn_Attention_8323646620215.

LayerNorm -> QKV -> scores(+rel-bias+mask) -> softmax -> attn@V -> out proj.

Sharding: 8 cores = (batch b in 0..3) x (query-half in 0..1). Each core
computes the full K/V for its batch and attention for its 1024 query rows;
no cross-core communication. Inside a core everything is computed in
transposed layouts so no on-chip transposes are needed beyond one PE
transpose of the LayerNorm output:

  xn_T[feat, tok]  (PE transpose)
  QT/KT[hd, tok] = Wqkv.T-slices @ xn_T      (scores contraction over hd=64)
  V[tok, hd]     = xn_T-slices.T @ Wqkv-v
  S_T[j, i]      = KT-slice.T @ QT           (psum, f32)
  P_T            = exp(S_T) * expA           (ACT exp, DVE mult)
  att_T[hd+1, i] = [V|1].T @ P_T             (row 64 = softmax denominator)
  y[i, :]        = sum_h outT_h-slice.T @ Wout-rows + bout

The mask+relative-bias enter multiplicatively: exp(s + bias + m) =
exp(s) * expA with expA = exp(clip-bias) * (mask != 0), and exp(clip-bias)
is generated on-chip from a tiny 3072-entry table via a flipped-identity
matmul (Toeplitz expansion). LN gamma / attention scale / beta are folded
into the QKV weights host-side (exact transforms).

The host permutes each core's tokens so its own query half comes first,
which keeps the device program identical across cores (single NEFF).
"""
import sys
import types
import numpy as np

sys.path.insert(0, "/opt/trn_rl_repo")

# ---- environment fixes (axon agent container) -------------------------------
if "antenv.axon_hooks" not in sys.modules:
    _m = types.ModuleType("antenv.axon_hooks")
    _m._hook = None
    _m.set_axon_ntff_profile_hook = lambda h: setattr(_m, "_hook", h)
    _m.get_axon_ntff_profile_hook = lambda: _m._hook
    sys.modules["antenv.axon_hooks"] = _m
    try:
        from trn_agent_boot.trn_boot import _ntff_profile_via_ctypes
        _m._hook = _ntff_profile_via_ctypes("/opt/axon/libaxon_pjrt.so")
    except Exception:
        pass

import ml_dtypes  # noqa: E402
from concourse import bass, mybir, tile  # noqa: E402
from concourse.bass_utils import run_bass_kernel_spmd  # noqa: E402
from concourse.masks import make_identity  # noqa: E402

F32 = mybir.dt.float32
BF16 = mybir.dt.bfloat16
AF = mybir.ActivationFunctionType
OP = mybir.AluOpType

B, N, D, H, DH, MAXREL = 4, 2048, 512, 8, 64, 200
NQ = N // 2          # queries per core
NT = N // 128        # 16 token tiles
NCORES = 8

# This container's walrus rejects instructions with more than one sem wait.
# Splitting is sound: a same-engine NoOp right before the instruction
# enforces the wait at the same program point (sequencers run in order).


def _split_waits(nc, maxw=1):
    n_split = 0
    for f in nc.m.functions:
        for blk in f.blocks:
            bb = blk.bb if hasattr(blk, "bb") else blk
            insts = list(bb.instructions)
            out = []
            changed = False
            for inst in insts:
                si = inst.sync_info
                waits = list(si.on_wait) if si and si.on_wait else []
                if len(waits) > maxw:
                    extra = waits[:-maxw]
                    chunks = [extra[j:j + maxw] for j in range(0, len(extra), maxw)]
                    for i, chunk in enumerate(chunks):
                        nop = mybir.InstNoOp(name=f"{inst.name}-ws{i}", ins=[], outs=[])
                        nop.engine = inst.engine
                        nop.sync_info = mybir.SyncInfo(on_wait=chunk, on_update=[])
                        out.append(nop)
                    si.on_wait = waits[-maxw:]
                    changed = True
                    n_split += 1
                out.append(inst)
            if changed:
                bb.instructions = out
    return n_split


def build(has_c=False):
    nc = bass.Bass("TRN2", target_bir_lowering=False, debug=False,
                   num_devices=NCORES)
    x_d = nc.dram_tensor("x", [N, D], F32, kind="ExternalInput")
    wqkv_d = nc.dram_tensor("wqkv", [D, 3 * D], BF16, kind="ExternalInput")
    cqkv_d = nc.dram_tensor("cqkv", [3 * D], F32, kind="ExternalInput")
    wout_d = nc.dram_tensor("wout", [D, D], BF16, kind="ExternalInput")
    bout_d = nc.dram_tensor("bout", [D], F32, kind="ExternalInput")
    m01t_d = nc.dram_tensor("m01t", [N, NQ], BF16, kind="ExternalInput")
    expa_d = nc.dram_tensor("expta", [3072], BF16, kind="ExternalInput")
    expb_d = nc.dram_tensor("exptb", [3072], BF16, kind="ExternalInput")
    jrev_d = nc.dram_tensor("jrev", [128, 128], BF16, kind="ExternalInput")
    dsb_d = nc.dram_tensor("den_scratch", [H, NQ], BF16)
    dsi_d = nc.dram_tensor("invden_scratch", [H, NQ], BF16)
    y_d = nc.dram_tensor("y", [NQ, D], F32, kind="ExternalOutput")

    with tile.TileContext(nc) as tc, \
         tc.tile_pool(name="const", bufs=1) as C, \
         tc.tile_pool(name="pers", bufs=1) as P, \
         tc.tile_pool(name="work", bufs=3) as W:

        # ---- persistent tiles ----------------------------------------------
        xnT = [P.tile([128, N], BF16, tag=f"xnT{fb}", name=f"xnT{fb}") for fb in range(4)]
        KTp = [P.tile([128, N], BF16, tag=f"KT{hp}", name=f"KT{hp}") for hp in range(4)]
        QTp = [P.tile([128, NQ], BF16, tag=f"QT{hp}", name=f"QT{hp}") for hp in range(4)]
        Vau = [P.tile([128, H, 66], BF16, tag=f"V{t}", name=f"Vau{t}") for t in range(NT)]
        expA = [P.tile([128, NQ], BF16, tag=f"eA{t}", name=f"eA{t}") for t in range(NT)]
        numT = [P.tile([65, NQ], BF16, tag=f"nT{h}", name=f"nT{h}") for h in range(H)]
        pairT = [P.tile([128, NQ], BF16, tag=f"pT{hp}", name=f"pT{hp}") for hp in range(4)]
        g_all = [P.tile([128, 2944], BF16, tag=f"gall{v}", name=f"gall{v}") for v in range(2)]

        # ---- input DMAs: x tiles first on sync (HWDGE), then mask tiles;
        # weights/tables ride the gpsimd (SWDGE) queues in parallel.
        x_ts = [W.tile([128, D], F32, tag="x", bufs=4, name=f"x{t}") for t in range(NT)]
        for t in range(NT):
            nc.sync.dma_start(out=x_ts[t][:], in_=x_d[t * 128:(t + 1) * 128, :])
        m_ts = [W.tile([128, NQ], BF16, tag="m01", bufs=4, name=f"m01_{t}") for t in range(NT)]
        for t in range(NT):
            nc.sync.dma_start(out=m_ts[t][:], in_=m01t_d[t * 128:(t + 1) * 128, :])

        ident = C.tile([128, 128], BF16, tag="ident")
        make_identity(nc, ident[:])
        jrev = C.tile([128, 128], BF16, tag="jrev")
        nc.gpsimd.dma_start(out=jrev[:], in_=jrev_d[:, :])
        eps_t = C.tile([128, 1], F32, tag="eps")
        nc.vector.memset(eps_t[:], 1e-5)
        wqkv_sb = [C.tile([128, 3 * D], BF16, tag=f"wqkv{kb}", name=f"wqkv{kb}") for kb in range(4)]
        for kb in range(4):
            nc.gpsimd.dma_start(out=wqkv_sb[kb][:],
                                in_=wqkv_d[kb * 128:(kb + 1) * 128, :])
        for v, tab in enumerate((expa_d, expb_d)):
            nc.gpsimd.dma_start(
                out=g_all[v][:],
                in_=bass.AP(tensor=tab.ap().tensor, offset=0,
                            ap=[[1, 128], [1, 2944]]))
        cq_all = C.tile([128, 12], F32, tag="cq")
        nc.gpsimd.dma_start(
            out=cq_all[:],
            in_=bass.AP(tensor=cqkv_d.ap().tensor, offset=0,
                        ap=[[1, 128], [128, 12]]))
        cv_bc = C.tile([128, D], F32, tag="cv")
        nc.gpsimd.dma_start(
            out=cv_bc[:],
            in_=bass.AP(tensor=cqkv_d.ap().tensor, offset=2 * D,
                        ap=[[0, 128], [1, D]]))
        bout_bc = C.tile([128, D], F32, tag="bout")
        nc.gpsimd.dma_start(
            out=bout_bc[:],
            in_=bass.AP(tensor=bout_d.ap().tensor, offset=0,
                        ap=[[0, 128], [1, D]]))
        woutP = [C.tile([128, D], BF16, tag=f"woutP{hp}", name=f"woutP{hp}") for hp in range(4)]
        for hp in range(4):
            nc.gpsimd.dma_start(out=woutP[hp][:],
                                in_=wout_d[hp * 128:(hp + 1) * 128, :])

        # ---- Phase A: LayerNorm + transpose, software-pipelined so the DVE
        # never sits in-FIFO behind the ACT sqrt round trip. -----------------
        with tc.tile_pool(name="psA", bufs=4, space="PSUM") as psA:
            mvs = [None] * NT
            rss = [None] * NT

            def ln_stats(t):
                st = W.tile([128, 6], F32, tag="st")
                nc.vector.bn_stats(out=st[:], in_=x_ts[t][:])
                mv = W.tile([128, 2], F32, tag="mv", bufs=4, name=f"mv{t}")
                nc.vector.bn_aggr(out=mv[:], in_=st[:])
                rs = W.tile([128, 1], F32, tag="rs", bufs=4, name=f"rs{t}")
                nc.scalar.activation(out=rs[:], in_=mv[:, 1:2], func=AF.Sqrt,
                                     bias=eps_t[:])
                mvs[t], rss[t] = mv, rs

            def ln_apply(t):
                nc.vector.reciprocal(out=rss[t][:], in_=rss[t][:])
                xn_bf = W.tile([128, D], BF16, tag="xn")
                nc.vector.tensor_scalar(out=xn_bf[:], in0=x_ts[t][:],
                                        scalar1=mvs[t][:, 0:1],
                                        scalar2=rss[t][:],
                                        op0=OP.subtract, op1=OP.mult)
                for fb in range(4):
                    tp = psA.tile([128, 128], BF16, tag="tr")
                    nc.tensor.transpose(tp[:], xn_bf[:, fb * 128:(fb + 1) * 128],
                                        ident[:])
                    if fb % 2 == 0:
                        nc.vector.tensor_copy(
                            out=xnT[fb][:, t * 128:(t + 1) * 128], in_=tp[:])
                    else:
                        nc.scalar.copy(
                            out=xnT[fb][:, t * 128:(t + 1) * 128], in_=tp[:])

            ln_stats(0)
            for t in range(1, NT):
                ln_stats(t)
                ln_apply(t - 1)
            ln_apply(NT - 1)

        # ---- Phase B: expA tiles first (PE+DVE), then QKV (PE+ACT/DVE) -----
        with tc.tile_pool(name="psB", bufs=2, space="PSUM") as psB:
            for t in range(NT):
                v = 0 if t < 8 else 1
                goff = 1920 - 128 * (t % 8)
                bp = psB.tile([128, NQ], F32, tag="bps")
                for ic in range(2):
                    nc.tensor.matmul(
                        bp[:, ic * 512:(ic + 1) * 512], jrev[:],
                        g_all[v][:, goff + ic * 512:goff + (ic + 1) * 512],
                        start=True, stop=True)
                nc.vector.tensor_mul(out=expA[t][:], in0=bp[:], in1=m_ts[t][:])
            for hp in range(4):
                for ic in range(2):
                    qp = psB.tile([128, 512], F32, tag="qkvps")
                    for kb in range(4):
                        nc.tensor.matmul(
                            qp[:],
                            wqkv_sb[kb][:, hp * 128:(hp + 1) * 128],
                            xnT[kb][:, ic * 512:(ic + 1) * 512],
                            start=(kb == 0), stop=(kb == 3))
                    if has_c:
                        nc.vector.tensor_scalar_add(
                            out=QTp[hp][:, ic * 512:(ic + 1) * 512], in0=qp[:],
                            scalar1=cq_all[:, hp:hp + 1])
                    else:
                        nc.scalar.copy(
                            out=QTp[hp][:, ic * 512:(ic + 1) * 512], in_=qp[:])
            for hp in range(4):
                for tc4 in range(4):
                    kp = psB.tile([128, 512], F32, tag="qkvps")
                    for kb in range(4):
                        nc.tensor.matmul(
                            kp[:],
                            wqkv_sb[kb][:, D + hp * 128:D + (hp + 1) * 128],
                            xnT[kb][:, tc4 * 512:(tc4 + 1) * 512],
                            start=(kb == 0), stop=(kb == 3))
                    if has_c:
                        nc.vector.tensor_scalar_add(
                            out=KTp[hp][:, tc4 * 512:(tc4 + 1) * 512], in0=kp[:],
                            scalar1=cq_all[:, 4 + hp:5 + hp])
                    else:
                        nc.scalar.copy(
                            out=KTp[hp][:, tc4 * 512:(tc4 + 1) * 512], in_=kp[:])
            for t in range(NT):
                vp = psB.tile([128, 512], F32, tag="qkvps")
                for kb in range(4):
                    nc.tensor.matmul(
                        vp[:],
                        xnT[kb][:, t * 128:(t + 1) * 128],
                        wqkv_sb[kb][:, 2 * D:3 * D],
                        start=(kb == 0), stop=(kb == 3))
                nc.vector.memset(Vau[t][:, :, 64:65], 1.0)
                if has_c:
                    nc.vector.tensor_add(out=Vau[t][:, :, 0:64], in0=vp[:],
                                         in1=cv_bc[:])
                else:
                    nc.vector.tensor_copy(out=Vau[t][:, :, 0:64], in_=vp[:])

        # ---- Phase C: attention --------------------------------------------
        def den_pieces(hp):
            """Normalization of pair hp's heads, as a list of small closures
            interleaved into the next pair's iterations (all DVE + DMA)."""
            dal = W.tile([128, 2 * NQ // 128], BF16, tag="dall", bufs=2,
                         name=f"dal{hp}")
            dbs = [None, None]

            def p0():
                nc.sync.dma_start(
                    out=dal[:],
                    in_=bass.AP(tensor=dsb_d.ap().tensor, offset=2 * hp * NQ,
                                ap=[[2 * NQ // 128, 128], [1, 2 * NQ // 128]]))

            def p1():
                nc.vector.tensor_scalar_add(out=dal[:], in0=dal[:],
                                            scalar1=1e-20)
                with nc.allow_low_precision(reason="bf16 softmax denominators"):
                    nc.vector.reciprocal(out=dal[:], in_=dal[:])
                nc.sync.dma_start(
                    out=bass.AP(tensor=dsi_d.ap().tensor, offset=2 * hp * NQ,
                                ap=[[2 * NQ // 128, 128], [1, 2 * NQ // 128]]),
                    in_=dal[:])

            def load_bc(e):
                def f():
                    h = 2 * hp + e
                    den_bc = W.tile([64, NQ], BF16, tag="denb", bufs=2,
                                    name=f"denb{h}")
                    dbs[e] = den_bc
                    nc.sync.dma_start(
                        out=den_bc[:],
                        in_=bass.AP(tensor=dsi_d.ap().tensor, offset=h * NQ,
                                    ap=[[0, 64], [1, NQ]]))
                return f

            def mul_chunk(e, half):
                def f():
                    h = 2 * hp + e
                    sl = slice(half * 512, (half + 1) * 512)
                    if e == 0:
                        nc.vector.tensor_mul(out=pairT[hp][0:64, sl],
                                             in0=numT[h][0:64, sl],
                                             in1=dbs[e][:, sl])
                    else:
                        nc.vector.tensor_mul(out=numT[h][0:64, sl],
                                             in0=numT[h][0:64, sl],
                                             in1=dbs[e][:, sl])
                return f

            def stitch():
                nc.sync.dma_start(out=pairT[hp][64:128, :],
                                  in_=numT[2 * hp + 1][0:64, :])

            return [p0, p1, load_bc(0), mul_chunk(0, 0), mul_chunk(0, 1),
                    load_bc(1), mul_chunk(1, 0), mul_chunk(1, 1), stitch]

        with tc.tile_pool(name="psC", bufs=2, space="PSUM") as psC:
            pend = []
            for hp in range(4):
                h0, h1 = 2 * hp, 2 * hp + 1
                av0 = psC.tile([65, NQ], F32, tag="av0", name="av0", bufs=1)
                av1 = psC.tile([65, NQ], F32, tag="av1", name="av1", bufs=1)
                for jt in range(NT):
                    for ic in range(2):
                        i5 = ic * 512
                        sp = psC.tile([128, 1024], F32, tag="sp")
                        nc.tensor.matmul(
                            sp[:, 0:512],
                            KTp[hp][0:64, jt * 128:(jt + 1) * 128],
                            QTp[hp][0:64, i5:i5 + 512],
                            start=True, stop=True, tile_position=(0, 0))
                        nc.tensor.matmul(
                            sp[:, 512:1024],
                            KTp[hp][64:128, jt * 128:(jt + 1) * 128],
                            QTp[hp][64:128, i5:i5 + 512],
                            start=True, stop=True, tile_position=(64, 0))
                        eb = W.tile([128, 1024], BF16, tag="eb")
                        nc.scalar.activation(out=eb[:], in_=sp[:], func=AF.Exp)
                        pb = W.tile([128, 1024], BF16, tag="pb")
                        nc.vector.tensor_mul(out=pb[:, 0:512], in0=eb[:, 0:512],
                                             in1=expA[jt][:, i5:i5 + 512])
                        nc.vector.tensor_mul(out=pb[:, 512:1024],
                                             in0=eb[:, 512:1024],
                                             in1=expA[jt][:, i5:i5 + 512])
                        nc.tensor.matmul(av0[:, i5:i5 + 512],
                                         Vau[jt][:, h0, 0:65], pb[:, 0:512],
                                         start=(jt == 0), stop=(jt == NT - 1))
                        nc.tensor.matmul(av1[:, i5:i5 + 512],
                                         Vau[jt][:, h1, 0:65], pb[:, 512:1024],
                                         start=(jt == 0), stop=(jt == NT - 1))
                        if pend and jt >= 2:
                            pend.pop(0)()
                        if jt == NT - 1:
                            nc.vector.tensor_copy(
                                out=numT[h0][:, i5:i5 + 512],
                                in_=av0[:, i5:i5 + 512])
                            nc.scalar.copy(
                                out=numT[h1][:, i5:i5 + 512],
                                in_=av1[:, i5:i5 + 512])
                for e in range(2):
                    h = 2 * hp + e
                    nc.sync.dma_start(out=dsb_d[h, :], in_=numT[h][64:65, :])
                pend = den_pieces(hp)
            for f in pend:
                f()

        # ---- Phase D: output projection (head pairs, K=128) ----------------
        with tc.tile_pool(name="psD", bufs=1, space="PSUM") as psD:
            yps = [psD.tile([128, 512], F32, tag=f"yp{isl}", name=f"yp{isl}")
                   for isl in range(8)]
            for hp in range(4):
                for isl in range(8):
                    nc.tensor.matmul(yps[isl][:],
                                     pairT[hp][:, isl * 128:(isl + 1) * 128],
                                     woutP[hp][:],
                                     start=(hp == 0), stop=(hp == 3))
            for isl in range(8):
                ysb = W.tile([128, 512], F32, tag="ysb", bufs=2)
                nc.vector.tensor_add(out=ysb[:], in0=yps[isl][:],
                                     in1=bout_bc[:])
                nc.sync.dma_start(out=y_d[isl * 128:(isl + 1) * 128, :],
                                  in_=ysb[:])
    _split_waits(nc)
    return nc


_NC_CACHE = {}


def _get_nc(has_c):
    if has_c not in _NC_CACHE:
        _NC_CACHE[has_c] = build(has_c)
    return _NC_CACHE[has_c]


LAST_EXEC_TIME_NS = None


def kernel(x, gamma, beta, Wqkv, Wout, bout, rel_table, temporal_mask,
           trace=True):
    global LAST_EXEC_TIME_NS
    x = np.asarray(x, np.float32)
    gamma = np.asarray(gamma, np.float32)
    beta = np.asarray(beta, np.float32)
    Wqkv = np.asarray(Wqkv, np.float32)
    Wout = np.asarray(Wout, np.float32)
    bout = np.asarray(bout, np.float32)
    rel_table = np.asarray(rel_table, np.float32)
    temporal_mask = np.asarray(temporal_mask)

    scale = DH ** -0.5
    w_eff = (Wqkv * gamma[:, None]).copy()
    w_eff[:, :D] *= scale
    cqkv = (beta @ Wqkv).astype(np.float32)
    cqkv[:D] *= scale
    wqkv_bf = w_eff.astype(ml_dtypes.bfloat16)
    wout_bf = Wout.astype(ml_dtypes.bfloat16)
    mask01T = (temporal_mask != 0).astype(np.float32).T  # [key j, query i]

    def expbias(d):
        idx = np.clip(d, -(MAXREL - 1), MAXREL - 1) + MAXREL - 1
        return np.exp(rel_table[idx])

    w = np.arange(3072)
    expt_a = expbias(w - 2047).astype(ml_dtypes.bfloat16)
    expt_b_half = [
        expbias(w - 2047 + 2 * (half * NQ) - NQ).astype(ml_dtypes.bfloat16)
        for half in range(2)
    ]
    keyperm_half = [
        np.concatenate([np.arange(i0, i0 + NQ),
                        np.arange(NQ - i0, NQ - i0 + NQ)])
        for i0 in (0, NQ)
    ]
    m01t_half = [
        np.ascontiguousarray(
            mask01T[keyperm_half[half]][:, half * NQ:(half + 1) * NQ]
        ).astype(ml_dtypes.bfloat16)
        for half in range(2)
    ]
    jrev_np = np.eye(128, dtype=np.float32)[::-1].astype(ml_dtypes.bfloat16).copy()

    in_maps = []
    for c in range(NCORES):
        b, half = c // 2, c % 2
        xp = np.ascontiguousarray(x[b][keyperm_half[half]])
        in_maps.append({
            "x": xp,
            "wqkv": wqkv_bf,
            "cqkv": cqkv,
            "wout": wout_bf,
            "bout": bout,
            "m01t": m01t_half[half],
            "expta": expt_a,
            "exptb": expt_b_half[half],
            "jrev": jrev_np,
        })

    nc = _get_nc(bool(np.any(cqkv != 0.0)))
    res = run_bass_kernel_spmd(nc, in_maps, core_ids=list(range(NCORES)),
                               trace=trace)
    LAST_EXEC_TIME_NS = res.exec_time_ns

    out = np.empty((B, N, D), np.float32)
    for c in range(NCORES):
        b, half = c // 2, c % 2
        out[b, half * NQ:(half + 1) * NQ] = res.results[c]["y"]
    return out
